# revision 27
# baseline (speedup 1.0000x reference)
"""Trainium2 Bass kernel for MinibatchDiscrimination (v3).

Reference computation (fp32):
    m = (x @ W.T + b).reshape(nb, 64, 16)            # nb=512
    d[i,j,B] = sum_c |m[i,B,c] - m[j,B,c]|
    o[i,B]   = sum_j exp(-d[i,j,B])
    out      = concat(x, o, axis=1)                   # (512, 1088)

Strategy (8 cores, 32-ring symmetric decomposition): core c owns output
rows R_c = [64c, 64c+64), x row-rotated per core.  Rows are split into
4 sub-blocks of 16; the 512 columns form a ring of 32 16-blocks.  A row
in 16-block r computes exp(-d) over the 272-column window of blocks
r..r+16 (local cols [16s, 16s+272) for sub-block s).  Row sums cover
distances 0..+16; per-column sums over window cols [16,256) (distances
+1..+15) are exported and added host-side to the owners of those rows
(by symmetry of d).  Distance-16 blocks are computed by both endpoint
blocks, each feeding only its own row sums -- no double count.

On-device pipeline per core:
  mT = W @ x^T + b as tiles [128 (B,c), 320 j]: fp8 DoubleRow matmuls
      (x, 64*W in fp8e4m3; PSUM scaled by 1/64 on evacuation).  Tiles
      0..5 evacuate to fp16 (DVE), tiles 6..7 to fp8 (ACT) for the
      fp8 pairwise path.
  Per row i: tiles 0..5 min-path on DVE (fp16, 4x mode); tile 6
      min-path on Pool (fp8); tile 7 abs-path on ACT (fp8).  The two
      fp8 results land in the halves of one [128,2,272] tile consumed
      by a single DoubleRow matmul (0.5 cyc/row; the odd row's DR uses
      a full-width zero-low lhsT since DR must target PSUM partition
      0); fp16 tiles use 6 plain matmuls.  A per-pair replicate matmul
      folds -S_j into the same PSUM accumulation, so ACT's Exp
      (bias=-S_i) directly yields exp(-d), written fp8 into couple
      halves with accum_out giving the row sums for free.
  colsum: one DoubleRow matmul per pair-couple over E[:,:,16:256],
      accumulated per sub-block into partition-disjoint regions of two
      PSUM banks (zero-half lhsT selects the region; later regions
      start from the accumulated zeros); evacuated once at the end.

Host assembles: out = concat(x, rowsums + scattered colsums, axis=1).

reps>1 unrolls the body for marginal (steady-state) timing.
The container's walrus rejects >1 sync wait per instruction; the
_split_multi_waits pass legalizes (hoists extras onto NoOps)."""

import os
import sys
import numpy as np

if "/opt/trn_rl_repo" not in sys.path:
    sys.path.insert(0, "/opt/trn_rl_repo")

NB = 512          # batch rows
NIN = 1024        # n_in
NBF = 64          # n_B
NCD = 16          # n_C
FOUT = NBF * NCD  # 1024 projection features
NCORES = 8
IB = NB // NCORES  # 64 output rows per core
TILEW = 320        # mt tile width (union of sub-block windows)
WIN = 272          # per-row j-window (17 x 16-blocks)
SUBW = 16          # sub-block row granularity
NSUB = IB // SUBW  # 4 sub-blocks per core
NPAIR = IB // 2    # 32 psd pair tiles
PPS = NPAIR // NSUB  # 8 pairs per sub-block
GROUP = 3          # pairs per psd PSUM group
EXLO, EXHI = 16, 256  # window-local col range exported as colsum partials
EXW = EXHI - EXLO     # 240
NF16 = 6           # tiles 0..5 -> fp16 DVE min path; 6 Pool fp8 min; 7 ACT fp8 abs
WSCALE = 64.0      # host premultiplies W by this; evac rescales by 1/WSCALE

_CACHE = {}


def _build_program(reps=1):
    import concourse.bass as bass
    import concourse.tile as tile
    from concourse import mybir
    from contextlib import ExitStack

    f32 = mybir.dt.float32
    f16 = mybir.dt.float16
    f8 = mybir.dt.float8e4
    Alu = mybir.AluOpType
    Act = mybir.ActivationFunctionType
    PM = mybir.MatmulPerfMode

    nc = bass.Bass()
    xTr_d = nc.declare_dram_parameter("xTr8", [NIN, TILEW], f8, isOutput=False)
    w8_d = nc.declare_dram_parameter("w8", [NIN, FOUT], f8, isOutput=False)
    b_d = nc.declare_dram_parameter("b", [FOUT], f32, isOutput=False)
    ind16_d = nc.declare_dram_parameter("ind16", [128, (NF16 + 1) * NBF], f16, isOutput=False)
    ind8dr_d = nc.declare_dram_parameter(
        "ind8dr", [128, 2 * NBF + 2 * 128 + NBF + 2 * 256], f8, isOutput=False)
    repl64_d = nc.declare_dram_parameter("repl64", [NBF, 128], f16, isOutput=False)
    o_d = nc.declare_dram_parameter("o", [128, NPAIR], f32, isOutput=True)
    cp_d = nc.declare_dram_parameter("cpart", [128, 2 * EXW], f32, isOutput=True)

    with tile.TileContext(nc) as tc, ExitStack() as ctx:
        singles = ctx.enter_context(tc.tile_pool(name="singles", bufs=1))
        scr16 = ctx.enter_context(tc.tile_pool(name="scr16", bufs=48))
        scr8 = ctx.enter_context(tc.tile_pool(name="scr8", bufs=14))
        epool = ctx.enter_context(tc.tile_pool(name="epool", bufs=10))
        psA = ctx.enter_context(tc.tile_pool(name="psA", bufs=2, space="PSUM"))
        psQ = ctx.enter_context(tc.tile_pool(name="psQ", bufs=1, space="PSUM"))
        psC = ctx.enter_context(tc.tile_pool(name="psC", bufs=1, space="PSUM"))
        psB = ctx.enter_context(tc.tile_pool(name="psB", bufs=GROUP + 1, space="PSUM"))

        dma = nc.default_dma_engine

        # ---- persistent loads -------------------------------------------
        # W in 4 big contiguous DMAs (k-major rows; per-partition lines are
        # two 1KB chunks), slab 0 and the x tiles first so the projection
        # starts before the whole stream lands
        wsl8 = []
        xr8 = []
        for kb2 in range(4):
            tl = singles.tile([128, 2, FOUT], f8, name=f"wsl{kb2}",
                              tag=f"wsl{kb2}")
            dma.dma_start(out=tl, in_=w8_d[256 * kb2:256 * (kb2 + 1), :]
                          .rearrange("(two p) c -> p two c", two=2))
            wsl8.append(tl)
            tx = singles.tile([128, 2, TILEW], f8, name=f"xr{kb2}",
                              tag=f"xr{kb2}")
            dma.dma_start(out=tx, in_=xTr_d[256 * kb2:256 * (kb2 + 1), :]
                          .rearrange("(two p) j -> p two j", two=2))
            xr8.append(tx)

        b_sb = singles.tile([128, 8], f32, name="b_sb", tag="b_sb")
        dma.dma_start(out=b_sb, in_=b_d.rearrange("(t p) -> p t", p=128))
        # f16 constants in one DMA: ind16 tiles (partition-major) + ones16
        c16 = singles.tile([128, (NF16 + 1) * NBF], f16, name="c16", tag="c16")
        dma.dma_start(out=c16, in_=ind16_d[:, :])
        ind16 = [c16[:, NBF * t:NBF * (t + 1)] for t in range(NF16)]
        ones16 = c16[:, NBF * NF16:NBF * (NF16 + 1)]
        # fp8 constants in one DMA: ind8dr (2x64) + ind8dr1 (2x128) + ind8s
        c8 = singles.tile([128, 2 * NBF + 2 * 128 + NBF + 2 * 256], f8,
                          name="c8", tag="c8")
        dma.dma_start(out=c8, in_=ind8dr_d[:, :])
        ind8dr = c8[:, 0:2 * NBF].rearrange("p (two c) -> p two c", two=2)
        ind8dr1 = c8[:, 2 * NBF:2 * NBF + 256].rearrange(
            "p (two c) -> p two c", two=2)
        ind8s = c8[:, 2 * NBF + 256:2 * NBF + 256 + NBF]
        q8 = 2 * NBF + 256 + NBF
        ones8lo = c8[:, q8:q8 + 256].rearrange("p (two c) -> p two c", two=2)
        ones8hi = c8[:, q8 + 256:q8 + 512].rearrange(
            "p (two c) -> p two c", two=2)
        repl64 = singles.tile([NBF, 128], f16, name="repl64", tag="repl64")
        dma.dma_start(out=repl64, in_=repl64_d[:, :])

        # persistent compute tiles (rewritten each rep)
        mt16 = [singles.tile([128, TILEW], f16, name=f"mt{t}", tag=f"mt{t}")
                for t in range(NF16)]
        mt8 = [singles.tile([128, TILEW], f8, name=f"mt8_{t}", tag=f"mt8_{t}")
               for t in (6, 7)]
        mc32 = [singles.tile([128, IB], f32, name=f"mc{t}", tag=f"mc{t}")
                for t in range(NF16)]
        mc8v = [singles.tile([128, IB], f32, name=f"mc8_{t}", tag=f"mc8_{t}")
                for t in (6, 7)]
        S16 = singles.tile([NBF, TILEW], f16, name="S16", tag="S16")
        negS2 = singles.tile([128, NPAIR], f32, name="negS2", tag="negS2")
        oacc = singles.tile([128, NPAIR], f32, name="oacc", tag="oacc")
        cp_sb = singles.tile([128, 2 * EXW], f32, name="cp", tag="cp")

        def one_rep():
            # ---- mT = W @ x^T (fp8 DoubleRow), evac (+b, /WSCALE) -------
            for t in range(8):
                ps = psA.tile([128, TILEW], f32, name="mps", tag="mps")
                for kb2 in range(4):
                    nc.tensor.matmul(
                        ps, lhsT=wsl8[kb2][:, :, 128 * t:128 * (t + 1)],
                        rhs=xr8[kb2],
                        start=(kb2 == 0), stop=(kb2 == 3),
                        perf_mode=PM.DoubleRow,
                    )
                if t < NF16:
                    nc.vector.tensor_scalar(
                        out=mt16[t], in0=ps, scalar1=1.0 / WSCALE,
                        op0=Alu.mult, scalar2=b_sb[:, t:t + 1], op1=Alu.add)
                    nc.scalar.activation(
                        out=mc32[t], in_=ps[:, 0:IB], func=Act.Identity,
                        bias=b_sb[:, t:t + 1], scale=1.0 / WSCALE)
                else:
                    k = t - NF16
                    nc.scalar.activation(
                        out=mt8[k], in_=ps, func=Act.Identity,
                        bias=b_sb[:, t:t + 1], scale=1.0 / WSCALE)
                    nc.scalar.activation(
                        out=mc8v[k], in_=mt8[k][:, 0:IB], func=Act.Copy,
                        bias=0.0, scale=1.0)

            # ---- psq = 2*S over min tiles (0..6); S16, negS2 ------------
            psq = psQ.tile([128, TILEW], f32, name="psq", tag="psq")
            for t in range(NF16):
                nc.tensor.matmul(
                    psq[0:NBF, :], lhsT=ind16[t], rhs=mt16[t],
                    start=(t == 0), stop=False)
            nc.tensor.matmul(
                psq[0:NBF, :], lhsT=ind8s, rhs=mt8[0],
                start=False, stop=True)
            psq_pairs = psq[0:NBF, 0:IB].rearrange("b (p two) -> b two p", two=2)
            nc.scalar.activation(
                out=negS2[0:NBF, :], in_=psq_pairs[:, 0, :],
                func=Act.Copy, bias=0.0, scale=-0.5)
            nc.scalar.activation(
                out=negS2[NBF:128, :], in_=psq_pairs[:, 1, :],
                func=Act.Copy, bias=0.0, scale=-0.5)
            nc.scalar.activation(
                out=S16, in_=psq[0:NBF, :], func=Act.Copy, bias=0.0, scale=1.0)

            # colsum accumulators: 4 partition-disjoint PSUM regions.
            # caccA holds sub-blocks 0 (p0:64) / 2 (p64:128); caccB (reusing
            # psq's bank, dead after S16/negS2) holds 1 / 3.
            caccA = psC.tile([128, EXW], f32, name="caccA", tag="caccA")
            caccB = psQ.tile([128, EXW], f32, name="caccB", tag="psq")

            epend = [None]

            # ---- pairwise loop: tile-outer over groups of GROUP pairs ---
            group_list = []
            for s_ in range(NSUB):
                base = PPS * s_
                group_list += [[base, base + 1, base + 2],
                               [base + 3, base + 4, base + 5],
                               [base + 6, base + 7]]
            for pairs in group_list:
                soff = 16 * (pairs[0] // PPS)  # window offset, same per group
                psd = {}
                for p in pairs:
                    psd[p] = psB.tile([128, WIN], f32, name="psd", tag="psd")

                # fp8 producers first (slow engines), DVE fp16 after
                mn8t = {}
                for p in pairs:
                    for h in range(2):
                        i = 2 * p + h
                        m8 = scr8.tile([128, 2, WIN], f8, name="mn8", tag="mn8")
                        nc.gpsimd.tensor_scalar_min(
                            m8[:, 0, :], mt8[0][:, soff:soff + WIN],
                            mc8v[0][:, i:i + 1])
                        nc.scalar.activation(
                            out=m8[:, 1, :], in_=mt8[1][:, soff:soff + WIN],
                            func=Act.Abs, bias=mc8v[1][:, i:i + 1], scale=-1.0)
                        mn8t[(p, h)] = m8

                for e in range(NF16):
                    for h in range(2):
                        for p in pairs:
                            i = 2 * p + h
                            mn = scr16.tile([128, WIN], f16, name="mn", tag="mn")
                            nc.vector.tensor_scalar_min(
                                mn, mt16[e][:, soff:soff + WIN],
                                mc32[e][:, i:i + 1])
                            nc.tensor.matmul(
                                psd[p][NBF * h:NBF * (h + 1), :],
                                lhsT=ind16[e], rhs=mn,
                                start=(e == 0), stop=False)
                for p in pairs:
                    nc.tensor.matmul(
                        psd[p][0:NBF, :], lhsT=ind8dr, rhs=mn8t[(p, 0)],
                        start=False, stop=False, perf_mode=PM.DoubleRow)
                for p in pairs:
                    nc.tensor.matmul(
                        psd[p][:, :], lhsT=ind8dr1, rhs=mn8t[(p, 1)],
                        start=False, stop=False, perf_mode=PM.DoubleRow,
                        skip_group_check=True)
                for p in pairs:
                    nc.tensor.matmul(
                        psd[p][:, :], lhsT=repl64,
                        rhs=S16[:, soff:soff + WIN],
                        start=False, stop=True, skip_group_check=True)

                for p in pairs:
                    s = p // PPS
                    if p % 2 == 0:
                        E = epool.tile([128, 2, WIN], f8, name="E", tag="E")
                        epend[0] = E
                    else:
                        E = epend[0]
                    nc.scalar.activation(
                        out=E[:, p % 2, :], in_=psd[p], func=Act.Exp,
                        bias=negS2[:, p:p + 1], scale=1.0,
                        accum_out=oacc[:, p:p + 1])
                    if p % 2 == 1:
                        # one DR colsum per couple; zero-half lhsT keeps the
                        # dst at partition 0 (adds +0 to the other region).
                        # Regions after the bank's first couple start from
                        # those accumulated zeros, so no start reset needed;
                        # stop only on the bank's final couple.
                        first = p in (1, PPS + 1)
                        last = p in (3 * PPS - 1, 4 * PPS - 1)
                        nc.tensor.matmul(
                            caccA[:, :] if s % 2 == 0 else caccB[:, :],
                            lhsT=ones8hi if s >= 2 else ones8lo,
                            rhs=E[:, :, EXLO:EXHI],
                            start=first, stop=last,
                            perf_mode=PM.DoubleRow, skip_group_check=True)

            dma.dma_start(out=o_d[:, :], in_=oacc)
            nc.vector.tensor_scalar_mul(out=cp_sb[:, 0:EXW], in0=caccA,
                                        scalar1=1.0)
            nc.vector.tensor_scalar_mul(out=cp_sb[:, EXW:2 * EXW], in0=caccB,
                                        scalar1=1.0)
            dma.dma_start(out=cp_d[:, :], in_=cp_sb)

        for _ in range(reps):
            one_rep()

    _dedup_ldweights(nc)
    _split_multi_waits(nc, mybir)
    return nc


def _dedup_ldweights(nc):
    """Drop PE InstLdweights whose weights AP + mode matches the previous
    load and which carry no sync (weights stay resident in the PE array).
    Self-loading matmuls (fp32/fp32r) clobber resident weights and
    invalidate the tracking."""
    f = nc.m.functions[0]
    n = 0
    for blk in f.blocks:
        last_key = None
        keep = []
        pend_waits = []
        for inst in blk.instructions:
            if str(inst.engine) == "EngineType.PE":
                if pend_waits:
                    si0 = inst.sync_info
                    if si0 is None:
                        import concourse.mybir as _mb
                        inst.sync_info = _mb.SyncInfo(
                            on_wait=list(pend_waits), on_update=[])
                    else:
                        si0.on_wait = list(si0.on_wait or []) + pend_waits
                    pend_waits = []
                tname = type(inst).__name__
                if tname == "InstLdweights":
                    si = inst.sync_info
                    clean = not si or (not si.on_wait and not si.on_update)
                    key = (
                        str(inst.ins[0]),
                        str(inst.perf_mode),
                        str(inst.is_transpose),
                        str(inst.tile_position),
                    )
                    if key == last_key and (
                        clean or not si.on_update
                    ):
                        # weights already resident: drop the load.  Waits
                        # migrate onto the following PE instruction (merged
                        # there; _split_multi_waits legalizes any overflow).
                        if not clean:
                            pend_waits.extend(si.on_wait)
                        n += 1
                        continue
                    last_key = key
                elif tname == "InstMatmult":
                    if inst.ldweights is not False:
                        last_key = None
            keep.append(inst)
        blk.instructions[:] = keep
    return n


def _split_multi_waits(nc, mybir):
    """Walrus here rejects >1 sync wait per instruction; hoist extras onto
    single-wait NoOps just before, on the same engine queue."""
    f = nc.m.functions[0]
    n_split = 0
    for blk in f.blocks:
        idx = 0
        while idx < len(blk.instructions):
            inst = blk.instructions[idx]
            si = inst.sync_info
            waits = list(si.on_wait) if si is not None and si.on_wait else []
            if len(waits) > 1:
                bysem = {}
                for w in waits:
                    k = w.id
                    if k not in bysem or (w.wait_value or 0) > (
                        bysem[k].wait_value or 0
                    ):
                        bysem[k] = w
                waits = list(bysem.values())
                for w in waits[:-1]:
                    nop = mybir.InstNoOp(
                        name=nc.get_next_instruction_name(), ins=[], outs=[]
                    )
                    nop.engine = inst.engine
                    nop.sync_info = mybir.SyncInfo(on_wait=[w], on_update=[])
                    blk.instructions.insert(idx, nop)
                    idx += 1
                    n_split += 1
                si.on_wait = [waits[-1]]
            idx += 1
    return n_split


def _get_program(reps=1):
    key = f"nc{reps}"
    if key not in _CACHE:
        _CACHE[key] = _build_program(reps)
    return _CACHE[key]


def _make_indicators():
    import ml_dtypes
    f8 = ml_dtypes.float8_e4m3fn
    p0 = np.arange(128)
    ind16 = np.zeros((128, (NF16 + 1) * NBF), dtype=np.float16)
    for t in range(NF16):
        ind16[p0, NBF * t + t * 8 + p0 // NCD] = 2.0
    p = np.arange(128)
    ind8 = np.zeros((128, 2 * NBF + 2 * 128 + NBF + 2 * 256), dtype=f8)
    ind8[p, 48 + p // NCD] = f8(2.0)              # ind8dr half 0: tile 6 (min)
    ind8[p, NBF + 56 + p // NCD] = f8(-1.0)       # ind8dr half 1: tile 7 (abs)
    q = 2 * NBF                                   # ind8dr1: cols 0:64 zero
    ind8[p, q + NBF + 48 + p // NCD] = f8(2.0)
    ind8[p, q + 128 + NBF + 56 + p // NCD] = f8(-1.0)
    ind8[p, q + 256 + 48 + p // NCD] = f8(2.0)    # ind8s
    q2 = q + 256 + NBF                            # ones8lo/hi DR colsum lhsT
    bb8 = np.arange(NBF)
    for half in range(2):                         # both halves same pattern
        ind8[bb8, q2 + 128 * half + bb8] = f8(1.0)          # lo: cols 0:64
        ind8[NBF + bb8, q2 + 128 * half + bb8] = f8(1.0)
        ind8[bb8, q2 + 256 + 128 * half + NBF + bb8] = f8(1.0)  # hi: cols 64:128
        ind8[NBF + bb8, q2 + 256 + 128 * half + NBF + bb8] = f8(1.0)
    bb = np.arange(NBF)
    ind16[bb, NBF * NF16 + bb] = 1.0       # ones16 block
    ind16[NBF + bb, NBF * NF16 + bb] = 1.0
    repl64 = np.zeros((NBF, 128), dtype=np.float16)
    repl64[bb, bb] = -0.5
    repl64[bb, NBF + bb] = -0.5
    return ind16, ind8, repl64


def make_in_maps(x, W, b):
    import ml_dtypes
    f8 = ml_dtypes.float8_e4m3fn
    x = np.ascontiguousarray(x, dtype=np.float32)
    W = np.ascontiguousarray(W, dtype=np.float32)
    b = np.ascontiguousarray(b, dtype=np.float32)
    ind16, ind8, repl64 = _make_indicators()

    w8 = (W.T * WSCALE).astype(f8)  # [NIN, FOUT], k-major

    in_maps = []
    for c in range(NCORES):
        xr = np.roll(x, -IB * c, axis=0)
        xTr8 = np.ascontiguousarray(xr.T[:, :TILEW]).astype(f8)
        in_maps.append({
            "xTr8": xTr8, "w8": w8, "b": b, "ind16": ind16,
            "ind8dr": ind8, "repl64": repl64,
        })
    return in_maps


def kernel(x, W, b):
    from concourse.bass_utils import run_bass_kernel_spmd

    x = np.ascontiguousarray(x, dtype=np.float32)
    nc = _get_program()
    in_maps = make_in_maps(x, W, b)

    res = run_bass_kernel_spmd(nc, in_maps, list(range(NCORES)), trace=False)
    _CACHE["last_results"] = res

    o_full = np.zeros((NB, NBF), dtype=np.float64)
    for c in range(NCORES):
        oc = np.asarray(res.results[c]["o"], dtype=np.float64)
        o_core = np.empty((IB, NBF))
        o_core[0::2, :] = oc[0:NBF, :].T
        o_core[1::2, :] = oc[NBF:128, :].T
        o_full[IB * c:IB * (c + 1), :] += o_core
        cp = np.asarray(res.results[c]["cpart"], dtype=np.float64)
        for s in range(NSUB):
            blk = cp[64 * (s // 2):64 * (s // 2) + NBF,
                     EXW * (s % 2):EXW * (s % 2 + 1)]
            rows = (IB * c + SUBW * s + EXLO + np.arange(EXW)) % NB
            o_full[rows, :] += blk.T
    return np.concatenate([x, o_full.astype(np.float32)], axis=1)
